# revision 21
# baseline (speedup 1.0000x reference)
"""Trainium2 Bass kernel for the CoverageMechanism (repeat-penalty) problem.

Reference semantics: for logits [B=4, S=512, V=32000] and generated_tokens
[B, S], the output is

    out[b, i, v] = logits[b, i, v] - 0.3 * #{j in [i-4, i) : tokens[b, j] == v}

for i >= 4, and out = logits for i < 4.  That is the identity on 262 MB of
logits plus an extremely sparse update: each (b, i) row of 32000 floats has
at most 4 elements decremented.

Strategy (8 NeuronCores, in-place sparse update — no bulk copy):
  - Flatten (b, i) to 2048 rows, shard 256 rows per core (the penalty
    window never crosses a batch row boundary, and the host has all the
    tokens, so no halo exchange is needed).
  - The per-core logits shard is passed as the *initial contents of the
    donated output buffer* (the same donated-operand mechanism
    run_bass_via_pjrt uses for its zero-initialized outputs, just
    initialized with the logits instead of zeros).  The device program
    therefore performs no 32.75 MB copy at all.
  - The device-side shard layout is VOCAB-MAJOR: [V=32000, R=256] f32
    (the host hands the donated buffer the transposed shard and
    transposes back after the run — a pure layout choice for the device
    buffer).  In this layout every penalty hitting vocab value v within
    the core lands in the single contiguous 256-float column v.  All
    tokens with the same value therefore MERGE into one scatter slot
    whose block id is v itself (< 32768, fits the scatter's int16
    index), so a core needs at most 259 slots (padded to 272) instead
    of the 1024 that the row-major layout forces (4 per row).  GPSIMD
    SWDGE descriptor generation is ~6-10 ns/slot and is the dominant
    serial cost, so this is the main lever.
  - Host preprocesses the 8 KB token tensor into per-core scatter-add
    metadata: int16 vocab ids + 256-float penalty columns holding
    -0.3*count (clipped to targets i in [4, 511], windows never cross
    the batch boundary).
  - Device program: load the 4.4 KB idx table and the 384 KB payload on
    the two otherwise-idle HWDGE queues (desc-gen only dereferences
    idx, so the first prep starts as soon as idx lands and the payload
    streams in underneath), generate two scatter windows (256 + 16
    slots) of CCE-add descriptors on GPSIMD, ring each doorbell as the
    prep commits, and read-modify-write only the penalized 1 KB columns
    in place; window 0's drain hides under window 1's desc-gen.
  - All 272 block ids within a window are unique (slots are distinct
    vocab values; padding slots point at unpenalized values with zero
    payload), so the CCE read-modify-write has no same-address races.
"""

import numpy as np
import jax

import concourse.bass as bass
import concourse.bacc as bacc
import concourse.mybir as mybir
import concourse.bass2jax as b2j
from concourse import library_config
from jax.sharding import Mesh, PartitionSpec
from jax.experimental.shard_map import shard_map

B, S, V = 4, 512, 32000
M = 4                      # sliding window length
W = np.float32(0.3)        # penalty weight
NCORES = 8
R = (B * S) // NCORES      # 256 rows per core
N = R * V                  # 8_192_000 f32 per core
ES = R                     # scatter elem_size: one 256-f32 column = 1 KB
KW = (272,)                # scatter window slots (>= 259 worst case)
K = sum(KW)                # 272 total slots
IDXC = K // 16             # idx columns (17)
# per-window payload groups of 128 slots (round_up(kw, 128) // 128)
PAYG = tuple(-(-k // 128) for k in KW)          # (2, 1)
PAYC = sum(PAYG) * ES      # payload columns (768)

_RT = None                 # cached (nc, run) runtime


def _build_bass():
    # Bacc (not raw Bass): its compile() pass auto-inserts the GPSIMD
    # library load that DMAScatterAddAnt needs.  The enlarged SWDGE
    # descriptor ring comfortably fits both preps.
    nc = bacc.Bacc("TRN2", target_bir_lowering=False,
                   dynamic_dma_scratch_size=65536)
    pay = nc.dram_tensor("pay", [128, PAYC], mybir.dt.float32,
                         kind="ExternalInput")
    idx = nc.dram_tensor("idx", [128, IDXC], mybir.dt.int16,
                         kind="ExternalInput")
    out = nc.dram_tensor("out", [N], mybir.dt.float32, kind="ExternalOutput")

    with (
        nc.sbuf_tensor("pay_sb", [128, PAYC], mybir.dt.float32) as pay_sb,
        nc.sbuf_tensor("idx_sb", [128, IDXC], mybir.dt.int16) as idx_sb,
        nc.semaphore("idx_sem") as idx_sem,
        nc.semaphore("pay_sem") as pay_sem,
        nc.semaphore("prep_sem") as prep_sem,
        nc.semaphore("sc_sem") as sc_sem,
    ):
        # Metadata loads on the two otherwise-idle HWDGE queues (issuing
        # them from Pool would put ~1.6 us of memcopy desc-gen on the Q7
        # critical path).  Desc-gen only dereferences idx_sb and its
        # ~2.5 us round trip gates everything, so BOTH queues race an
        # identical idx load into the same SBUF bytes — wait_ge(16) is
        # satisfied by whichever lands first, trimming the head's
        # tail-latency jitter.  The 384 KB payload follows on the scalar
        # queue; it streams in under the prep and is only awaited before
        # the doorbell.
        nc.sync.dma_start(idx_sb[:, :], idx[:, :]).then_inc(idx_sem, 16)
        nc.scalar.dma_start(idx_sb[:, :], idx[:, :]).then_inc(idx_sem, 16)
        nc.scalar.dma_start(pay_sb[:, :], pay[:, :]).then_inc(pay_sem, 16)

        # Hoist the Q7 library switch (DMAScatterAddAnt lives in `mlp`)
        # above the idx wait so the ~0.5 us reload overlaps the metadata
        # DMA latency instead of delaying the first desc-gen.
        nc.gpsimd.load_library(library_config.mlp)

        # All slots across both windows target distinct vocab values, so
        # the two windows' in-flight CCE-adds can never collide; window
        # 0's doorbell rings while window 1's desc-gen runs, hiding most
        # of window 0's drain.
        out_win = out.ap().rearrange("(a b) -> a b", b=ES)  # [32000, 256]
        nc.gpsimd.wait_ge(idx_sem, 16)
        ic = pc = 0
        for w, kw in enumerate(KW):
            pay_ap = pay_sb[:, pc * ES:(pc + PAYG[w]) * ES].rearrange(
                "p (g e) -> p g e", e=ES)                   # [128, g, 256]
            idx_ap = idx_sb[:, ic:ic + kw // 16]            # [128, kw/16]
            nc.gpsimd.dma_scatter_add(
                out_win, pay_ap, idx_ap, kw, kw, ES,
                prepare_only=True, sem=sc_sem,
            ).then_inc(prep_sem, 1)
            # Ring window w's doorbell right after its desc-gen commits
            # (triggering before the Q7 finishes races the ring and
            # wedges the device) and the payload is resident.
            nc.gpsimd.wait_ge(prep_sem, w + 1)
            if w == 0:
                nc.gpsimd.wait_ge(pay_sem, 16)
            nc.gpsimd.trigger_dma(count=1)
            ic += kw // 16
            pc += PAYG[w]
        # The sc_sem wait is load-bearing: the walrus end-of-NEFF drain
        # chains RESET DMA state rather than fencing it — without this
        # wait the in-flight scatter gets cancelled and no penalties land.
        nc.gpsimd.wait_ge(sc_sem, 16 * len(KW))
    nc.compile()
    return nc


def _make_runner(nc, n_cores):
    """jit-compiled SPMD executor for `nc` with the output buffer
    initialized from a donated operand (run_bass_via_pjrt's mechanism,
    with caller-controlled initial contents instead of zeros)."""
    b2j.install_neuronx_cc_hook()
    partition_name = (nc.partition_id_tensor.name
                      if nc.partition_id_tensor else None)
    in_names, out_names, out_avals = [], [], []
    for alloc in nc.m.functions[0].allocations:
        if not isinstance(alloc, mybir.MemoryLocationSet):
            continue
        name = alloc.memorylocations[0].name
        if alloc.kind == "ExternalInput":
            if name != partition_name:
                in_names.append(name)
        elif alloc.kind == "ExternalOutput":
            out_names.append(name)
            out_avals.append(jax.core.ShapedArray(
                tuple(alloc.tensor_shape), mybir.dt.np(alloc.dtype)))
    n_params = len(in_names)
    all_in_names = in_names + out_names
    if partition_name is not None:
        all_in_names.append(partition_name)

    def _body(*args):
        operands = list(args)
        if partition_name is not None:
            operands.append(b2j.partition_id_tensor())
        outs = b2j._bass_exec_p.bind(
            *operands,
            out_avals=tuple(out_avals),
            in_names=tuple(all_in_names),
            out_names=tuple(out_names),
            lowering_input_output_aliases=(),
            sim_require_finite=True,
            sim_require_nnan=True,
            nc=nc,
        )
        return tuple(outs)

    devices = jax.devices()[:n_cores]
    mesh = Mesh(np.asarray(devices), ("core",))
    spec = PartitionSpec("core")
    sharded = jax.jit(
        shard_map(_body, mesh=mesh,
                  in_specs=(spec,) * (n_params + len(out_names)),
                  out_specs=(spec,) * len(out_names),
                  check_rep=False),
        donate_argnums=tuple(range(n_params, n_params + len(out_names))),
        keep_unused=True,
    )

    def run(in_maps, out_inits):
        concat_in = [
            np.concatenate([np.asarray(in_maps[c][nm]) for c in range(n_cores)],
                           axis=0)
            for nm in in_names
        ]
        outs = sharded(*concat_in, *out_inits)
        return [np.asarray(o).reshape(n_cores, *a.shape)
                for o, a in zip(outs, out_avals)]

    return run


def _get_runtime():
    global _RT
    if _RT is None:
        nc = _build_bass()
        _RT = (nc, _make_runner(nc, NCORES))
    return _RT


def _preprocess(tokens):
    """tokens [B, S] -> per-core scatter payload/index arrays.

    Returns (pay [8, 128, PAYC] f32, idx [8, 128, IDXC] int16).

    The device shard of core c is vocab-major [V, R]: element (v, r) is
    logits[flat row c*R + r, v].  Slot k holds the full 256-float penalty
    column for one distinct vocab value v: col[r] = -0.3 * (number of
    window tokens with value v for flat row c*R + r).  Its block id IS v
    (the [V, R] view makes column v the v-th 256-float block), stored at
    idx[k%16 + 16m, colof(k)] for the 8 replica groups m; the payload
    lives at pay[k%128, group(k)*ES : +ES], with windows of KW slots
    laid out consecutively.  Padding slots point at unpenalized vocab
    values with zero payload, so all block ids are unique -> no RMW
    races.
    """
    tokens = np.asarray(tokens).astype(np.int64)
    flat = tokens.reshape(B * S)
    pay_all = np.zeros((NCORES, 128, PAYC), np.float32)
    idx_all = np.zeros((NCORES, 128, IDXC), np.int16)
    for c in range(NCORES):
        pay, idx = pay_all[c], idx_all[c]
        base = c * R
        cols: dict[int, np.ndarray] = {}
        # token at flat position j penalizes flat rows j+1..j+4, clipped
        # to the same batch and to in-batch position >= M
        for j in range(max(base - M, 0), min(base + R - 1, B * S - 1)):
            b, i = divmod(j, S)
            lo = max(i + 1, M)
            hi = min(i + M, S - 1)
            if lo > hi:
                continue
            v = int(flat[j])
            col = cols.get(v)
            if col is None:
                col = cols[v] = np.zeros(R, np.float32)
            for t in range(lo, hi + 1):
                r = b * S + t - base
                if 0 <= r < R:
                    col[r] -= W
        assert len(cols) <= K
        entries = sorted(cols.items())
        used = set(cols)
        t = 0
        while len(entries) < K:
            if t not in used:
                entries.append((t, None))
            t += 1
        k = ic = pc = 0
        for w, kw in enumerate(KW):
            for kk in range(kw):
                v, col = entries[k]
                k += 1
                idx[kk % 16::16, ic + kk // 16] = v
                if col is not None:
                    g = pc + kk // 128
                    pay[kk % 128, g * ES:(g + 1) * ES] = col
            ic += kw // 16
            pc += PAYG[w]
    return pay_all, idx_all


def kernel(logits, generated_tokens):
    logits = np.asarray(logits, dtype=np.float32)
    pay_all, idx_all = _preprocess(generated_tokens)
    in_maps = [{"pay": pay_all[c], "idx": idx_all[c]} for c in range(NCORES)]
    # device shards are vocab-major [V, R] per core
    out_init = np.ascontiguousarray(
        logits.reshape(NCORES, R, V).transpose(0, 2, 1)).reshape(NCORES * N)
    _, run = _get_runtime()
    outs = run(in_maps, [out_init])
    return np.ascontiguousarray(
        outs[0].reshape(NCORES, V, R).transpose(0, 2, 1)).reshape(B, S, V)


# revision 22
# speedup vs baseline: 1.0843x; 1.0843x over previous
"""Trainium2 Bass kernel for the CoverageMechanism (repeat-penalty) problem.

Reference semantics: for logits [B=4, S=512, V=32000] and generated_tokens
[B, S], the output is

    out[b, i, v] = logits[b, i, v] - 0.3 * #{j in [i-4, i) : tokens[b, j] == v}

for i >= 4, and out = logits for i < 4.  That is the identity on 262 MB of
logits plus an extremely sparse update: each (b, i) row of 32000 floats has
at most 4 elements decremented.

Strategy (8 NeuronCores, in-place sparse update — no bulk copy):
  - Flatten (b, i) to 2048 rows, shard 256 rows per core (the penalty
    window never crosses a batch row boundary, and the host has all the
    tokens, so no halo exchange is needed).
  - The per-core logits shard is passed as the *initial contents of the
    donated output buffer* (the same donated-operand mechanism
    run_bass_via_pjrt uses for its zero-initialized outputs, just
    initialized with the logits instead of zeros).  The device program
    therefore performs no 32.75 MB copy at all.
  - The device-side shard layout is VOCAB-MAJOR: [V=32000, R=256] f32
    (the host hands the donated buffer the transposed shard and
    transposes back after the run — a pure layout choice for the device
    buffer).  In this layout every penalty hitting vocab value v within
    the core lands in the single contiguous 256-float column v.  All
    tokens with the same value therefore MERGE into one scatter slot
    whose block id is v itself (< 32768, fits the scatter's int16
    index), so a core needs at most 259 slots (padded to 272) instead
    of the 1024 that the row-major layout forces (4 per row).  GPSIMD
    SWDGE descriptor generation is ~6-10 ns/slot and is the dominant
    serial cost, so this is the main lever.
  - Host preprocesses the 8 KB token tensor into per-core scatter-add
    metadata: int16 vocab ids + 256-float penalty columns holding
    -0.3*count (clipped to targets i in [4, 511], windows never cross
    the batch boundary).
  - Device program: load the 4.4 KB idx table and the 384 KB payload on
    the two otherwise-idle HWDGE queues (desc-gen only dereferences
    idx, so the first prep starts as soon as idx lands and the payload
    streams in underneath), generate two scatter windows (256 + 16
    slots) of CCE-add descriptors on GPSIMD, ring each doorbell as the
    prep commits, and read-modify-write only the penalized 1 KB columns
    in place; window 0's drain hides under window 1's desc-gen.
  - All 272 block ids within a window are unique (slots are distinct
    vocab values; padding slots point at unpenalized values with zero
    payload), so the CCE read-modify-write has no same-address races.
"""

import numpy as np
import jax

import concourse.bass as bass
import concourse.bacc as bacc
import concourse.mybir as mybir
import concourse.bass2jax as b2j
from concourse import library_config
from jax.sharding import Mesh, PartitionSpec
from jax.experimental.shard_map import shard_map

B, S, V = 4, 512, 32000
M = 4                      # sliding window length
W = np.float32(0.3)        # penalty weight
NCORES = 8
R = (B * S) // NCORES      # 256 rows per core
N = R * V                  # 8_192_000 f32 per core
ES = R                     # scatter elem_size: one 256-f32 column = 1 KB
KW = (256, 16)             # slots per scatter window (sum >= 259 worst case)
K = sum(KW)                # 272 total slots
IDXC = K // 16             # idx columns (17)
# per-window payload groups of 128 slots (round_up(kw, 128) // 128)
PAYG = tuple(-(-k // 128) for k in KW)          # (2, 1)
PAYC = sum(PAYG) * ES      # payload columns (768)

_RT = None                 # cached (nc, run) runtime


def _build_bass():
    # Bacc (not raw Bass): its compile() pass auto-inserts the GPSIMD
    # library load that DMAScatterAddAnt needs.  The enlarged SWDGE
    # descriptor ring comfortably fits both preps.
    nc = bacc.Bacc("TRN2", target_bir_lowering=False,
                   dynamic_dma_scratch_size=65536)
    pay = nc.dram_tensor("pay", [128, PAYC], mybir.dt.float32,
                         kind="ExternalInput")
    idx = nc.dram_tensor("idx", [128, IDXC], mybir.dt.int16,
                         kind="ExternalInput")
    out = nc.dram_tensor("out", [N], mybir.dt.float32, kind="ExternalOutput")

    with (
        nc.sbuf_tensor("pay_sb", [128, PAYC], mybir.dt.float32) as pay_sb,
        nc.sbuf_tensor("idx_sb", [128, IDXC], mybir.dt.int16) as idx_sb,
        nc.semaphore("idx_sem") as idx_sem,
        nc.semaphore("pay_sem") as pay_sem,
        nc.semaphore("prep_sem") as prep_sem,
        nc.semaphore("sc_sem") as sc_sem,
    ):
        # Metadata loads on the two otherwise-idle HWDGE queues (issuing
        # them from Pool would put ~1.6 us of memcopy desc-gen on the Q7
        # critical path).  Desc-gen only dereferences idx_sb and its
        # ~2.5 us round trip gates everything, so BOTH queues race an
        # identical idx load into the same SBUF bytes — wait_ge(16) is
        # satisfied by whichever lands first, trimming the head's
        # tail-latency jitter.  The 384 KB payload follows on the scalar
        # queue; it streams in under the prep and is only awaited before
        # the doorbell.
        nc.sync.dma_start(idx_sb[:, :], idx[:, :]).then_inc(idx_sem, 16)
        nc.scalar.dma_start(idx_sb[:, :], idx[:, :]).then_inc(idx_sem, 16)
        nc.scalar.dma_start(pay_sb[:, :], pay[:, :]).then_inc(pay_sem, 16)

        # Hoist the Q7 library switch (DMAScatterAddAnt lives in `mlp`)
        # above the idx wait so the ~0.5 us reload overlaps the metadata
        # DMA latency instead of delaying the first desc-gen.
        nc.gpsimd.load_library(library_config.mlp)

        # All slots across both windows target distinct vocab values, so
        # the two windows' in-flight CCE-adds can never collide; window
        # 0's doorbell rings while window 1's desc-gen runs, hiding most
        # of window 0's drain.
        out_win = out.ap().rearrange("(a b) -> a b", b=ES)  # [32000, 256]
        nc.gpsimd.wait_ge(idx_sem, 16)
        ic = pc = 0
        for w, kw in enumerate(KW):
            pay_ap = pay_sb[:, pc * ES:(pc + PAYG[w]) * ES].rearrange(
                "p (g e) -> p g e", e=ES)                   # [128, g, 256]
            idx_ap = idx_sb[:, ic:ic + kw // 16]            # [128, kw/16]
            nc.gpsimd.dma_scatter_add(
                out_win, pay_ap, idx_ap, kw, kw, ES,
                prepare_only=True, sem=sc_sem,
            ).then_inc(prep_sem, 1)
            # Ring window w's doorbell right after its desc-gen commits
            # (triggering before the Q7 finishes races the ring and
            # wedges the device) and the payload is resident.
            nc.gpsimd.wait_ge(prep_sem, w + 1)
            if w == 0:
                nc.gpsimd.wait_ge(pay_sem, 16)
            nc.gpsimd.trigger_dma(count=1)
            ic += kw // 16
            pc += PAYG[w]
        # The sc_sem wait is load-bearing: the walrus end-of-NEFF drain
        # chains RESET DMA state rather than fencing it — without this
        # wait the in-flight scatter gets cancelled and no penalties land.
        nc.gpsimd.wait_ge(sc_sem, 16 * len(KW))
    nc.compile()
    return nc


def _make_runner(nc, n_cores):
    """jit-compiled SPMD executor for `nc` with the output buffer
    initialized from a donated operand (run_bass_via_pjrt's mechanism,
    with caller-controlled initial contents instead of zeros)."""
    b2j.install_neuronx_cc_hook()
    partition_name = (nc.partition_id_tensor.name
                      if nc.partition_id_tensor else None)
    in_names, out_names, out_avals = [], [], []
    for alloc in nc.m.functions[0].allocations:
        if not isinstance(alloc, mybir.MemoryLocationSet):
            continue
        name = alloc.memorylocations[0].name
        if alloc.kind == "ExternalInput":
            if name != partition_name:
                in_names.append(name)
        elif alloc.kind == "ExternalOutput":
            out_names.append(name)
            out_avals.append(jax.core.ShapedArray(
                tuple(alloc.tensor_shape), mybir.dt.np(alloc.dtype)))
    n_params = len(in_names)
    all_in_names = in_names + out_names
    if partition_name is not None:
        all_in_names.append(partition_name)

    def _body(*args):
        operands = list(args)
        if partition_name is not None:
            operands.append(b2j.partition_id_tensor())
        outs = b2j._bass_exec_p.bind(
            *operands,
            out_avals=tuple(out_avals),
            in_names=tuple(all_in_names),
            out_names=tuple(out_names),
            lowering_input_output_aliases=(),
            sim_require_finite=True,
            sim_require_nnan=True,
            nc=nc,
        )
        return tuple(outs)

    devices = jax.devices()[:n_cores]
    mesh = Mesh(np.asarray(devices), ("core",))
    spec = PartitionSpec("core")
    sharded = jax.jit(
        shard_map(_body, mesh=mesh,
                  in_specs=(spec,) * (n_params + len(out_names)),
                  out_specs=(spec,) * len(out_names),
                  check_rep=False),
        donate_argnums=tuple(range(n_params, n_params + len(out_names))),
        keep_unused=True,
    )

    def run(in_maps, out_inits):
        concat_in = [
            np.concatenate([np.asarray(in_maps[c][nm]) for c in range(n_cores)],
                           axis=0)
            for nm in in_names
        ]
        outs = sharded(*concat_in, *out_inits)
        return [np.asarray(o).reshape(n_cores, *a.shape)
                for o, a in zip(outs, out_avals)]

    return run


def _get_runtime():
    global _RT
    if _RT is None:
        nc = _build_bass()
        _RT = (nc, _make_runner(nc, NCORES))
    return _RT


def _preprocess(tokens):
    """tokens [B, S] -> per-core scatter payload/index arrays.

    Returns (pay [8, 128, PAYC] f32, idx [8, 128, IDXC] int16).

    The device shard of core c is vocab-major [V, R]: element (v, r) is
    logits[flat row c*R + r, v].  Slot k holds the full 256-float penalty
    column for one distinct vocab value v: col[r] = -0.3 * (number of
    window tokens with value v for flat row c*R + r).  Its block id IS v
    (the [V, R] view makes column v the v-th 256-float block), stored at
    idx[k%16 + 16m, colof(k)] for the 8 replica groups m; the payload
    lives at pay[k%128, group(k)*ES : +ES], with windows of KW slots
    laid out consecutively.  Padding slots point at unpenalized vocab
    values with zero payload, so all block ids are unique -> no RMW
    races.
    """
    tokens = np.asarray(tokens).astype(np.int64)
    flat = tokens.reshape(B * S)
    pay_all = np.zeros((NCORES, 128, PAYC), np.float32)
    idx_all = np.zeros((NCORES, 128, IDXC), np.int16)
    for c in range(NCORES):
        pay, idx = pay_all[c], idx_all[c]
        base = c * R
        cols: dict[int, np.ndarray] = {}
        # token at flat position j penalizes flat rows j+1..j+4, clipped
        # to the same batch and to in-batch position >= M
        for j in range(max(base - M, 0), min(base + R - 1, B * S - 1)):
            b, i = divmod(j, S)
            lo = max(i + 1, M)
            hi = min(i + M, S - 1)
            if lo > hi:
                continue
            v = int(flat[j])
            col = cols.get(v)
            if col is None:
                col = cols[v] = np.zeros(R, np.float32)
            for t in range(lo, hi + 1):
                r = b * S + t - base
                if 0 <= r < R:
                    col[r] -= W
        assert len(cols) <= K
        entries = sorted(cols.items())
        used = set(cols)
        t = 0
        while len(entries) < K:
            if t not in used:
                entries.append((t, None))
            t += 1
        k = ic = pc = 0
        for w, kw in enumerate(KW):
            for kk in range(kw):
                v, col = entries[k]
                k += 1
                idx[kk % 16::16, ic + kk // 16] = v
                if col is not None:
                    g = pc + kk // 128
                    pay[kk % 128, g * ES:(g + 1) * ES] = col
            ic += kw // 16
            pc += PAYG[w]
    return pay_all, idx_all


def kernel(logits, generated_tokens):
    logits = np.asarray(logits, dtype=np.float32)
    pay_all, idx_all = _preprocess(generated_tokens)
    in_maps = [{"pay": pay_all[c], "idx": idx_all[c]} for c in range(NCORES)]
    # device shards are vocab-major [V, R] per core
    out_init = np.ascontiguousarray(
        logits.reshape(NCORES, R, V).transpose(0, 2, 1)).reshape(NCORES * N)
    _, run = _get_runtime()
    outs = run(in_maps, [out_init])
    return np.ascontiguousarray(
        outs[0].reshape(NCORES, V, R).transpose(0, 2, 1)).reshape(B, S, V)
